# revision 2
# baseline (speedup 1.0000x reference)
import sys
import types

import numpy as np

B, H, W, K = 8, 512, 512, 8
PB = 150000                 # points per block (= per cloud)
P = B * PB
NPART = 128
NCOLS = 1172                # 128*1172 = 150016 padded rows per block
RPAD = NPART * NCOLS
PADROW = PB                 # local row id of the white pad point
SLOTS = 2048                # pixel slots per partition
CAP = NPART * SLOTS         # 262144 pixels per core
NPIX = B * H * W            # 2097152 == 8*CAP

TRACE = False
LAST_EXEC_NS = None

_PLANES = ("px", "py", "pz", "cx", "cy", "cz", "nx", "ny", "nz", "fr", "fg", "fb")


def _install_shim():
    try:
        from antenv.axon_hooks import get_axon_ntff_profile_hook  # noqa: F401
        return
    except Exception:
        pass
    try:
        import antenv
        from trn_agent_boot.trn_boot import _ntff_profile_via_ctypes

        mod = types.ModuleType("antenv.axon_hooks")
        _state = {"hook": _ntff_profile_via_ctypes("/opt/axon/libaxon_pjrt.so")}
        mod.set_axon_ntff_profile_hook = lambda h: _state.__setitem__("hook", h)
        mod.get_axon_ntff_profile_hook = lambda: _state["hook"]
        sys.modules["antenv.axon_hooks"] = mod
        antenv.axon_hooks = mod
    except Exception:
        pass


_prog = None


def _build_program():
    from concourse import bass, bacc, mybir
    import concourse.tile as tile

    f32 = mybir.dt.float32
    i32 = mybir.dt.int32
    A = mybir.AluOpType

    nc = bacc.Bacc()
    pl = {n: nc.declare_dram_parameter(n, [NPART, NCOLS], f32, False) for n in _PLANES}
    consts = nc.declare_dram_parameter("consts", [NPART, 8], f32, False)
    itd = nc.declare_dram_parameter("it", [NPART, SLOTS], i32, False)
    y = nc.declare_dram_parameter("y", [NPART, SLOTS, 4], f32, True)
    table = nc.dram_tensor("shaded_table", [RPAD, 4], f32, kind="Internal")

    with tile.TileContext(nc) as tc:
        with tc.tile_pool(name="p", bufs=1) as pool:
            sb = {n: pool.tile([NPART, NCOLS], f32, name=f"sb_{n}") for n in _PLANES}
            cs = pool.tile([NPART, 8], f32)
            it_sb = pool.tile([NPART, SLOTS], i32)
            for n in _PLANES:
                nc.sync.dma_start(out=sb[n][:], in_=pl[n][:])
            nc.sync.dma_start(out=cs[:], in_=consts[:])
            nc.sync.dma_start(out=it_sb[:], in_=itd[:])

            v = nc.vector
            lx, ly, lz = (cs[:, i : i + 1] for i in range(3))

            ndl = pool.tile([NPART, NCOLS], f32)
            t0 = pool.tile([NPART, NCOLS], f32)
            coef = pool.tile([NPART, NCOLS], f32)
            len2 = pool.tile([NPART, NCOLS], f32)
            ln = pool.tile([NPART, NCOLS], f32)
            inv = pool.tile([NPART, NCOLS], f32)

            # ndl = nx*lx + ny*ly + nz*lz
            v.tensor_scalar(out=ndl[:], in0=sb["nx"][:], scalar1=lx, scalar2=None, op0=A.mult)
            v.tensor_scalar(out=t0[:], in0=sb["ny"][:], scalar1=ly, scalar2=None, op0=A.mult)
            v.tensor_tensor(out=ndl[:], in0=ndl[:], in1=t0[:], op=A.add)
            v.tensor_scalar(out=t0[:], in0=sb["nz"][:], scalar1=lz, scalar2=None, op0=A.mult)
            v.tensor_tensor(out=ndl[:], in0=ndl[:], in1=t0[:], op=A.add)
            # coef = max(ndl,0)*0.7 + 0.3
            v.tensor_scalar(out=coef[:], in0=ndl[:], scalar1=0.0, scalar2=0.7, op0=A.max, op1=A.mult)
            v.tensor_scalar(out=coef[:], in0=coef[:], scalar1=0.3, scalar2=None, op0=A.add)
            # d = cam - p (into cam planes)
            for c, p_ in (("cx", "px"), ("cy", "py"), ("cz", "pz")):
                v.tensor_tensor(out=sb[c][:], in0=sb[c][:], in1=sb[p_][:], op=A.subtract)
            # len2 = |d|^2 ; inv = 1/sqrt(len2)
            v.tensor_tensor(out=len2[:], in0=sb["cx"][:], in1=sb["cx"][:], op=A.mult)
            v.tensor_tensor(out=t0[:], in0=sb["cy"][:], in1=sb["cy"][:], op=A.mult)
            v.tensor_tensor(out=len2[:], in0=len2[:], in1=t0[:], op=A.add)
            v.tensor_tensor(out=t0[:], in0=sb["cz"][:], in1=sb["cz"][:], op=A.mult)
            v.tensor_tensor(out=len2[:], in0=len2[:], in1=t0[:], op=A.add)
            nc.scalar.sqrt(out=ln[:], in_=len2[:])
            v.reciprocal(out=inv[:], in_=ln[:])
            # h = l + d*inv (into cam planes)
            for c, lc in (("cx", lx), ("cy", ly), ("cz", lz)):
                v.tensor_tensor(out=sb[c][:], in0=sb[c][:], in1=inv[:], op=A.mult)
                v.tensor_scalar(out=sb[c][:], in0=sb[c][:], scalar1=lc, scalar2=None, op0=A.add)
            # len2 = |h|^2 ; inv = 1/sqrt
            v.tensor_tensor(out=len2[:], in0=sb["cx"][:], in1=sb["cx"][:], op=A.mult)
            v.tensor_tensor(out=t0[:], in0=sb["cy"][:], in1=sb["cy"][:], op=A.mult)
            v.tensor_tensor(out=len2[:], in0=len2[:], in1=t0[:], op=A.add)
            v.tensor_tensor(out=t0[:], in0=sb["cz"][:], in1=sb["cz"][:], op=A.mult)
            v.tensor_tensor(out=len2[:], in0=len2[:], in1=t0[:], op=A.add)
            nc.scalar.sqrt(out=ln[:], in_=len2[:])
            v.reciprocal(out=inv[:], in_=ln[:])
            # ndh = max((n . h) * inv, 0) -> reuse ndl
            v.tensor_tensor(out=ndl[:], in0=sb["nx"][:], in1=sb["cx"][:], op=A.mult)
            v.tensor_tensor(out=t0[:], in0=sb["ny"][:], in1=sb["cy"][:], op=A.mult)
            v.tensor_tensor(out=ndl[:], in0=ndl[:], in1=t0[:], op=A.add)
            v.tensor_tensor(out=t0[:], in0=sb["nz"][:], in1=sb["cz"][:], op=A.mult)
            v.tensor_tensor(out=ndl[:], in0=ndl[:], in1=t0[:], op=A.add)
            v.tensor_tensor(out=ndl[:], in0=ndl[:], in1=inv[:], op=A.mult)
            v.tensor_scalar(out=ndl[:], in0=ndl[:], scalar1=0.0, scalar2=None, op0=A.max)
            # specs = 0.2 * ndh^32
            for _ in range(5):
                v.tensor_tensor(out=ndl[:], in0=ndl[:], in1=ndl[:], op=A.mult)
            v.tensor_scalar(out=ndl[:], in0=ndl[:], scalar1=0.2, scalar2=None, op0=A.mult)
            # shaded channels, interleaved [p, j, 4]
            shdA = pool.tile([NPART, NCOLS, 4], f32)
            for ch, f in enumerate(("fr", "fg", "fb")):
                v.tensor_tensor(out=t0[:], in0=sb[f][:], in1=coef[:], op=A.mult)
                v.tensor_tensor(out=shdA[:, :, ch], in0=t0[:], in1=ndl[:], op=A.add)
                v.tensor_scalar(
                    out=shdA[:, :, ch], in0=shdA[:, :, ch],
                    scalar1=0.0, scalar2=1.0, op0=A.max, op1=A.min,
                )
            # write table: row r = p*NCOLS + j
            tab_ap = table[:].rearrange("(p j) c -> p j c", j=NCOLS)
            nc.sync.dma_start(out=tab_ap, in_=shdA[:])

            tc.strict_bb_all_engine_barrier()

            gt = pool.tile([NPART, SLOTS, 4], f32)
            for j in range(SLOTS):
                nc.gpsimd.indirect_dma_start(
                    out=gt[:, j, :],
                    out_offset=None,
                    in_=table[:],
                    in_offset=bass.IndirectOffsetOnAxis(ap=it_sb[:, j : j + 1], axis=0),
                )
            nc.sync.dma_start(out=y[:], in_=gt[:])
    nc.finalize()
    return nc


def kernel(**inputs):
    global _prog, LAST_EXEC_NS
    _install_shim()
    from concourse.bass_utils import run_bass_kernel_spmd

    idx = np.asarray(inputs["idx"])
    points = np.ascontiguousarray(np.asarray(inputs["points"], dtype=np.float32))
    features = np.ascontiguousarray(np.asarray(inputs["features"], dtype=np.float32))
    normals = np.ascontiguousarray(np.asarray(inputs["normals"], dtype=np.float32))
    cam_centers = np.asarray(inputs["cam_centers"], dtype=np.float32)
    cloud_idx = np.asarray(inputs["cloud_idx"])
    light_dir = np.asarray(inputs["light_dir"], dtype=np.float32)

    l = (light_dir / max(np.sqrt(np.sum(light_dir * light_dir)), 1e-12)).astype(np.float32)
    campp = cam_centers[cloud_idx]  # [P,3] per-point camera

    idx0 = idx[..., 0].reshape(-1).astype(np.int64)
    key = np.where(idx0 >= 0, idx0 // PB, 8).astype(np.int8)
    order = np.argsort(key, kind="stable")
    counts = np.bincount(key, minlength=9)
    starts = np.concatenate([[0], np.cumsum(counts)])
    assert counts[:8].max() <= CAP, f"core overflow: {counts}"
    bg_pool = order[starts[8] :]

    consts_arr = np.zeros((NPART, 8), np.float32)
    consts_arr[:, 0:3] = l[None, :]

    pad = {
        "px": 0.0, "py": 0.0, "pz": 0.0,
        "cx": l[0], "cy": l[1], "cz": l[2],
        "nx": l[0], "ny": l[1], "nz": l[2],
        "fr": 1.0, "fg": 1.0, "fb": 1.0,
    }
    cols = {
        "px": points[:, 0], "py": points[:, 1], "pz": points[:, 2],
        "cx": campp[:, 0], "cy": campp[:, 1], "cz": campp[:, 2],
        "nx": normals[:, 0], "ny": normals[:, 1], "nz": normals[:, 2],
        "fr": features[:, 0], "fg": features[:, 1], "fb": features[:, 2],
    }

    in_maps = []
    pids = []
    bgpos = 0
    for c in range(8):
        own = order[starts[c] : starts[c] + counts[c]]
        need = CAP - counts[c]
        fill = bg_pool[bgpos : bgpos + need]
        bgpos += need
        pid = np.concatenate([own, fill])
        rows = np.full(CAP, PADROW, np.int32)
        rows[: counts[c]] = (idx0[own] - c * PB).astype(np.int32)
        pids.append(pid)

        m = {"consts": consts_arr, "it": rows.reshape(NPART, SLOTS)}
        lo = c * PB
        for n in _PLANES:
            buf = np.empty(RPAD, np.float32)
            buf[:PB] = cols[n][lo : lo + PB]
            buf[PB:] = pad[n]
            m[n] = buf.reshape(NPART, NCOLS)
        in_maps.append(m)
    assert bgpos == len(bg_pool)

    if _prog is None:
        _prog = _build_program()

    res = run_bass_kernel_spmd(_prog, in_maps, list(range(8)), trace=TRACE)
    if TRACE:
        LAST_EXEC_NS = res.exec_time_ns

    img_flat = np.empty((NPIX, 3), np.float32)
    for c in range(8):
        yc = np.asarray(res.results[c]["y"], np.float32).reshape(CAP, 4)
        img_flat[pids[c]] = yc[:, :3]
    return img_flat.reshape(B, H, W, 3)


# revision 3
# speedup vs baseline: 16.5155x; 16.5155x over previous
import sys
import types

import numpy as np

B, H, W, K = 8, 512, 512, 8
PB = 150000                 # points per block (= per cloud)
P = B * PB
NPART = 128
NCOLS = 1176                # table columns; col 0 = white, 1..1175 usable
NPIX = B * H * W            # 2097152

TRACE = False
LAST_EXEC_NS = None

_PLANES = ("px", "py", "pz", "cx", "cy", "cz", "nx", "ny", "nz", "fr", "fg", "fb")


def _install_shim():
    try:
        from antenv.axon_hooks import get_axon_ntff_profile_hook  # noqa: F401
        return
    except Exception:
        pass
    try:
        import antenv
        from trn_agent_boot.trn_boot import _ntff_profile_via_ctypes

        mod = types.ModuleType("antenv.axon_hooks")
        _state = {"hook": _ntff_profile_via_ctypes("/opt/axon/libaxon_pjrt.so")}
        mod.set_axon_ntff_profile_hook = lambda h: _state.__setitem__("hook", h)
        mod.get_axon_ntff_profile_hook = lambda: _state["hook"]
        sys.modules["antenv.axon_hooks"] = mod
        antenv.axon_hooks = mod
    except Exception:
        pass


_prog_cache = {}


def _build_program(ni):
    from concourse import bacc, mybir
    from concourse.library_config import ap_gather as apg_lib
    import concourse.tile as tile

    f32 = mybir.dt.float32
    i16 = mybir.dt.int16
    A = mybir.AluOpType

    nc = bacc.Bacc()
    pl = {n: nc.declare_dram_parameter(n, [NPART, NCOLS], f32, False) for n in _PLANES}
    consts = nc.declare_dram_parameter("consts", [NPART, 8], f32, False)
    gidxd = nc.declare_dram_parameter("gidx", [NPART, ni // 16], i16, False)
    y = nc.declare_dram_parameter("y", [NPART, ni, 4], f32, True)

    with tile.TileContext(nc) as tc:
        with tc.tile_pool(name="p", bufs=1) as pool:
            sb = {n: pool.tile([NPART, NCOLS], f32, name=f"sb_{n}") for n in _PLANES}
            cs = pool.tile([NPART, 8], f32)
            ix = pool.tile([NPART, ni // 16], i16)
            for n in _PLANES:
                nc.sync.dma_start(out=sb[n][:], in_=pl[n][:])
            nc.sync.dma_start(out=cs[:], in_=consts[:])
            nc.sync.dma_start(out=ix[:], in_=gidxd[:])
            nc.gpsimd.load_library(apg_lib)

            v = nc.vector
            lx, ly, lz = (cs[:, i : i + 1] for i in range(3))

            ndl = pool.tile([NPART, NCOLS], f32)
            t0 = pool.tile([NPART, NCOLS], f32)
            coef = pool.tile([NPART, NCOLS], f32)
            len2 = pool.tile([NPART, NCOLS], f32)
            ln = pool.tile([NPART, NCOLS], f32)
            inv = pool.tile([NPART, NCOLS], f32)

            # ndl = nx*lx + ny*ly + nz*lz
            v.tensor_scalar(out=ndl[:], in0=sb["nx"][:], scalar1=lx, scalar2=None, op0=A.mult)
            v.tensor_scalar(out=t0[:], in0=sb["ny"][:], scalar1=ly, scalar2=None, op0=A.mult)
            v.tensor_tensor(out=ndl[:], in0=ndl[:], in1=t0[:], op=A.add)
            v.tensor_scalar(out=t0[:], in0=sb["nz"][:], scalar1=lz, scalar2=None, op0=A.mult)
            v.tensor_tensor(out=ndl[:], in0=ndl[:], in1=t0[:], op=A.add)
            # coef = max(ndl,0)*0.7 + 0.3
            v.tensor_scalar(out=coef[:], in0=ndl[:], scalar1=0.0, scalar2=0.7, op0=A.max, op1=A.mult)
            v.tensor_scalar(out=coef[:], in0=coef[:], scalar1=0.3, scalar2=None, op0=A.add)
            # d = cam - p (into cam planes)
            for c, p_ in (("cx", "px"), ("cy", "py"), ("cz", "pz")):
                v.tensor_tensor(out=sb[c][:], in0=sb[c][:], in1=sb[p_][:], op=A.subtract)
            # len2 = |d|^2 ; inv = 1/sqrt(len2)
            v.tensor_tensor(out=len2[:], in0=sb["cx"][:], in1=sb["cx"][:], op=A.mult)
            v.tensor_tensor(out=t0[:], in0=sb["cy"][:], in1=sb["cy"][:], op=A.mult)
            v.tensor_tensor(out=len2[:], in0=len2[:], in1=t0[:], op=A.add)
            v.tensor_tensor(out=t0[:], in0=sb["cz"][:], in1=sb["cz"][:], op=A.mult)
            v.tensor_tensor(out=len2[:], in0=len2[:], in1=t0[:], op=A.add)
            nc.scalar.sqrt(out=ln[:], in_=len2[:])
            v.reciprocal(out=inv[:], in_=ln[:])
            # h = l + d*inv (into cam planes)
            for c, lc in (("cx", lx), ("cy", ly), ("cz", lz)):
                v.tensor_tensor(out=sb[c][:], in0=sb[c][:], in1=inv[:], op=A.mult)
                v.tensor_scalar(out=sb[c][:], in0=sb[c][:], scalar1=lc, scalar2=None, op0=A.add)
            # len2 = |h|^2 ; inv = 1/sqrt
            v.tensor_tensor(out=len2[:], in0=sb["cx"][:], in1=sb["cx"][:], op=A.mult)
            v.tensor_tensor(out=t0[:], in0=sb["cy"][:], in1=sb["cy"][:], op=A.mult)
            v.tensor_tensor(out=len2[:], in0=len2[:], in1=t0[:], op=A.add)
            v.tensor_tensor(out=t0[:], in0=sb["cz"][:], in1=sb["cz"][:], op=A.mult)
            v.tensor_tensor(out=len2[:], in0=len2[:], in1=t0[:], op=A.add)
            nc.scalar.sqrt(out=ln[:], in_=len2[:])
            v.reciprocal(out=inv[:], in_=ln[:])
            # ndh = max((n . h) * inv, 0) -> reuse ndl
            v.tensor_tensor(out=ndl[:], in0=sb["nx"][:], in1=sb["cx"][:], op=A.mult)
            v.tensor_tensor(out=t0[:], in0=sb["ny"][:], in1=sb["cy"][:], op=A.mult)
            v.tensor_tensor(out=ndl[:], in0=ndl[:], in1=t0[:], op=A.add)
            v.tensor_tensor(out=t0[:], in0=sb["nz"][:], in1=sb["cz"][:], op=A.mult)
            v.tensor_tensor(out=ndl[:], in0=ndl[:], in1=t0[:], op=A.add)
            v.tensor_tensor(out=ndl[:], in0=ndl[:], in1=inv[:], op=A.mult)
            v.tensor_scalar(out=ndl[:], in0=ndl[:], scalar1=0.0, scalar2=None, op0=A.max)
            # specs = 0.2 * ndh^32
            for _ in range(5):
                v.tensor_tensor(out=ndl[:], in0=ndl[:], in1=ndl[:], op=A.mult)
            v.tensor_scalar(out=ndl[:], in0=ndl[:], scalar1=0.2, scalar2=None, op0=A.mult)
            # shaded channels, interleaved [p, j, 4]
            shdA = pool.tile([NPART, NCOLS, 4], f32)
            for ch, f in enumerate(("fr", "fg", "fb")):
                v.tensor_tensor(out=t0[:], in0=sb[f][:], in1=coef[:], op=A.mult)
                v.tensor_tensor(out=shdA[:, :, ch], in0=t0[:], in1=ndl[:], op=A.add)
                v.tensor_scalar(
                    out=shdA[:, :, ch], in0=shdA[:, :, ch],
                    scalar1=0.0, scalar2=1.0, op0=A.max, op1=A.min,
                )

            tc.strict_bb_all_engine_barrier()

            gt = pool.tile([NPART, ni, 4], f32)
            nc.gpsimd.ap_gather(
                out_ap=gt[:], in_ap=shdA[:], idxs_ap=ix[:],
                channels=NPART, num_elems=NCOLS, d=4, num_idxs=ni,
            )

            tc.strict_bb_all_engine_barrier()

            nc.sync.dma_start(out=y[:], in_=gt[:])
    nc.finalize()
    return nc


def kernel(**inputs):
    global LAST_EXEC_NS
    _install_shim()
    from concourse.bass_utils import run_bass_kernel_spmd

    idx = np.asarray(inputs["idx"])
    points = np.ascontiguousarray(np.asarray(inputs["points"], dtype=np.float32))
    features = np.ascontiguousarray(np.asarray(inputs["features"], dtype=np.float32))
    normals = np.ascontiguousarray(np.asarray(inputs["normals"], dtype=np.float32))
    cam_centers = np.asarray(inputs["cam_centers"], dtype=np.float32)
    cloud_idx = np.asarray(inputs["cloud_idx"])
    light_dir = np.asarray(inputs["light_dir"], dtype=np.float32)

    l = (light_dir / max(np.sqrt(np.sum(light_dir * light_dir)), 1e-12)).astype(np.float32)
    campp = cam_centers[cloud_idx]  # [P,3] per-point camera

    idx0 = idx[..., 0].reshape(-1).astype(np.int64)
    key = np.where(idx0 >= 0, idx0 // PB, 8).astype(np.int8)
    order = np.argsort(key, kind="stable")
    kcounts = np.bincount(key, minlength=9)
    kstarts = np.concatenate([[0], np.cumsum(kcounts)])
    bg_pool = order[kstarts[8]:]
    nbg = len(bg_pool)
    bg_bounds = [nbg * c // 8 for c in range(9)]

    consts_arr = np.zeros((NPART, 8), np.float32)
    consts_arr[:, 0:3] = l[None, :]

    pad = {
        "px": 0.0, "py": 0.0, "pz": 0.0,
        "cx": l[0], "cy": l[1], "cz": l[2],
        "nx": l[0], "ny": l[1], "nz": l[2],
        "fr": 1.0, "fg": 1.0, "fb": 1.0,
    }
    cols = {
        "px": points[:, 0], "py": points[:, 1], "pz": points[:, 2],
        "cx": campp[:, 0], "cy": campp[:, 1], "cz": campp[:, 2],
        "nx": normals[:, 0], "ny": normals[:, 1], "nz": normals[:, 2],
        "fr": features[:, 0], "fg": features[:, 1], "fb": features[:, 2],
    }

    # pass 1: per-core placement (multiplicity-sorted blocks of 16 -> columns)
    place = []
    ni_need = 0
    for c in range(8):
        own = order[kstarts[c] : kstarts[c] + kcounts[c]]
        r = (idx0[own] - c * PB).astype(np.int64)
        counts = np.bincount(r, minlength=PB)
        ord_pts = np.argsort(-counts, kind="stable")
        U = int((counts > 0).sum())
        ord_pts = ord_pts[:U]
        m_s = counts[ord_pts]
        nb = (U + 15) // 16
        # block b -> group b%8, column 1 + b//8
        M = m_s[0::16].astype(np.int64)           # per-block slot demand
        S_all = np.zeros(nb, np.int64)
        T = np.zeros(8, np.int64)
        for g in range(8):
            mg = M[g::8]
            S_all[g::8] = np.cumsum(mg) - mg
            T[g] = mg.sum()
        # pixel -> (partition, slot)
        ipos = np.empty(PB, np.int64)
        ipos[ord_pts] = np.arange(U)
        i_pix = ipos[r]
        ordpix = np.argsort(i_pix, kind="stable")
        i_sorted = i_pix[ordpix]
        pstart = np.concatenate([[0], np.cumsum(m_s)])
        t_sorted = np.arange(len(own)) - np.repeat(pstart[:-1], m_s)
        blk = i_sorted // 16
        parts = 16 * (blk % 8) + (i_sorted % 16)
        slots = S_all[blk] + t_sorted
        # bg for this core, split g::8 within core
        bg_core = bg_pool[bg_bounds[c] : bg_bounds[c + 1]]
        bg_lens = np.array([len(bg_core[g::8]) for g in range(8)], np.int64)
        need = int((T + (bg_lens + 15) // 16).max())
        ni_need = max(ni_need, need)
        place.append((own, ord_pts, U, m_s, nb, M, S_all, T,
                      parts, slots, ordpix, bg_core))

    ni = max(((ni_need + 63) // 64) * 64, 64)

    # pass 2: build per-core device inputs + pid maps
    in_maps = []
    pids = []
    for c in range(8):
        (own, ord_pts, U, m_s, nb, M, S_all, T,
         parts, slots, ordpix, bg_core) = place[c]
        gidx = np.zeros((NPART, ni // 16), np.int16)
        pid = np.full((NPART, ni), -1, np.int64)
        for g in range(8):
            cols_g = 1 + (np.arange(g, nb, 8) // 8)
            seq = np.repeat(cols_g, M[g::8]).astype(np.int16)
            s = np.arange(T[g])
            gidx[16 * g + s % 16, s // 16] = seq
            ids = bg_core[g::8]
            i = np.arange(len(ids))
            pid[16 * g + i % 16, T[g] + i // 16] = ids
        pid[parts, slots] = own[ordpix]
        pids.append(pid)

        iu = np.arange(U)
        part_u = 16 * ((iu // 16) % 8) + iu % 16
        col_u = 1 + (iu // 16) // 8
        m = {"consts": consts_arr, "gidx": gidx}
        lo = c * PB
        for n in _PLANES:
            buf = np.full((NPART, NCOLS), pad[n], np.float32)
            buf[part_u, col_u] = cols[n][lo + ord_pts]
            m[n] = buf
        in_maps.append(m)

    prog = _prog_cache.get(ni)
    if prog is None:
        prog = _prog_cache[ni] = _build_program(ni)

    res = run_bass_kernel_spmd(prog, in_maps, list(range(8)), trace=TRACE)
    if TRACE:
        LAST_EXEC_NS = res.exec_time_ns

    img_flat = np.empty((NPIX, 3), np.float32)
    for c in range(8):
        yc = np.asarray(res.results[c]["y"], np.float32).reshape(NPART, ni, 4)
        pid = pids[c]
        sel = pid >= 0
        img_flat[pid[sel]] = yc[sel][:, :3]
    return img_flat.reshape(B, H, W, 3)
